# revision 44
# baseline (speedup 1.0000x reference)
"""Causal self-attention (L=8192, D=2048) on 8 TRN2 NeuronCores.

v2 design (single-shot latency optimized):

Sharding: core c owns 8 interleaved 128-row query tiles (slot s -> global row
tile 8s+c, i.e. rows where (r//128) % 8 == c) and KV rows [c*1024,(c+1)*1024).
The slot structure makes the static per-slot causal chunk count (4s+4 chunks
of 256 cols) load-balanced across cores: 288 (128x128) score tiles per core
vs the 260 exact-causal minimum and 384 for 512-row block interleaving.

Phase 1 projects K^T, V, Q^T locally (f32r matmuls, bf16 outputs).  K^T and V
are AllGathered in bf16 (halves collective bytes vs f32).  The z transpose is
split so the K projection (and therefore the K gather, the critical-path
collective) starts as early as possible; K/V stores ride the gpsimd DMA queue
so the SP load queue never head-blocks.  The V gather is issued after phase
2a's kt_g loads (the tile framework would otherwise serialize those loads
behind it) and runs while scores compute.

Phase 2a computes all causal scores S^T = K^T-tiles @ Q^T (gated only on
AG(K)), exp's them into a bf16 stash (pt_all), applies diagonal masks, and
accumulates the softmax denominators with 2-col ones-matmuls in PSUM.
Phase 2b (gated on AG(V)) computes P^T @ V in two dv-halves by chunk groups
with PSUM-resident accumulation within each group and one DVE add per
(slot, group, dvc); slot g finalizes (scale by 1/l, +bv, store) right after
its last contribution in group g.

When all biases are zero (as in this problem's inputs), kernel() compiles a
specialized program without the bias loads/adds; nonzero biases fall back to
the general build (cache keyed on the flag).
"""

import math
import time
from contextlib import ExitStack

import numpy as np

import concourse.bass as bass
import concourse.tile as tile
from concourse import bacc, mybir
from concourse.bass_utils import run_bass_kernel_spmd
from concourse.masks import make_identity

L = 8192
D = 2048  # d_x == d_attn == d_v
NCORES = 8
NSLOT = 8  # 128-row query tiles per core
JC = 256  # kv columns per chunk
NCH = L // JC  # 32 column chunks
NDT = D // 128  # 16 contraction tiles
SCALE = 1.0 / math.sqrt(D)

F32 = mybir.dt.float32
F32R = mybir.dt.float32r
BF16 = mybir.dt.bfloat16

_cache = {}


def _build(repeat=1, zero_bias=False):
    nc = bacc.Bacc("TRN2", num_devices=NCORES)

    x = nc.dram_tensor("x_rows", [1024, D], F32, kind="ExternalInput")
    z = nc.dram_tensor("z_blk", [1024, D], F32, kind="ExternalInput")
    wq = nc.dram_tensor("wq", [D, D], F32, kind="ExternalInput")
    wk = nc.dram_tensor("wk", [D, D], F32, kind="ExternalInput")
    wv = nc.dram_tensor("wv", [D, D], F32, kind="ExternalInput")
    bq = nc.dram_tensor("bq", [D], F32, kind="ExternalInput")
    bk = nc.dram_tensor("bk", [D], F32, kind="ExternalInput")
    bv = nc.dram_tensor("bv", [D], F32, kind="ExternalInput")
    ig_rows = nc.dram_tensor("ig_rows", [NSLOT * 128], F32, kind="ExternalInput")
    out = nc.dram_tensor("out", [1024, D], F32, kind="ExternalOutput")

    kt_loc = nc.dram_tensor("kt_loc", [4, 128, NDT, JC], BF16)
    v_loc = nc.dram_tensor("v_loc", [1024, D], BF16)
    kt_g = nc.dram_tensor("kt_g", [NCH, 128, NDT, JC], BF16, addr_space="Shared")
    v_g = nc.dram_tensor("v_g", [L, D], BF16, addr_space="Shared")

    with tile.TileContext(nc) as tc:
        with ExitStack() as consts:
            cp = consts.enter_context(tc.tile_pool(name="consts", bufs=1))
            ident = cp.tile([128, 128], F32)
            make_identity(nc, ident)
            ones = cp.tile([128, 2], BF16)
            nc.vector.memset(ones, 1.0)
            # jg[p, k] = 128*k + p == global j index of partition p in j-tile k
            jg = cp.tile([128, L // 128], F32)
            nc.gpsimd.iota(
                jg,
                pattern=[[128, L // 128]],
                base=0,
                channel_multiplier=1,
                allow_small_or_imprecise_dtypes=True,
            )
            # ig_sb[p, s, i] = global row index of col i of slot s (all p)
            ig_sb = cp.tile([128, NSLOT, 128], F32)
            nc.gpsimd.dma_start(
                ig_sb,
                bass.AP(tensor=ig_rows, offset=0, ap=[[0, 128], [1, NSLOT * 128]]),
            )
            bv_bc = cp.tile([128, D], F32)
            nc.gpsimd.dma_start(
                bv_bc, bass.AP(tensor=bv, offset=0, ap=[[0, 128], [1, D]])
            )
            bq_sb = cp.tile([128, NDT], F32, tag="bq")
            nc.gpsimd.dma_start(
                bq_sb, bass.AP(tensor=bq, offset=0, ap=[[1, 128], [128, NDT]])
            )
            bk_sb = cp.tile([128, NDT], F32, tag="bk")
            nc.gpsimd.dma_start(
                bk_sb, bass.AP(tensor=bk, offset=0, ap=[[1, 128], [128, NDT]])
            )

            for _rep in range(repeat):
                _one_rep(nc, tc, ident, ones, jg, ig_sb, bv_bc, bq_sb, bk_sb,
                         x, z, wq, wk, wv, kt_loc, v_loc, kt_g, v_g, out,
                         zero_bias)

    nc.finalize()
    return nc


def _one_rep(nc, tc, ident, ones, jg, ig_sb, bv_bc, bq_sb, bk_sb,
             x, z, wq, wk, wv, kt_loc, v_loc, kt_g, v_g, out, zero_bias=False):
    # pt_all column offsets: chunk m, j-tile jt -> [off, off+ifree)
    ifree = [1024 - 128 * (m // 4) for m in range(NCH)]
    pt_off = []
    o = 0
    for m in range(NCH):
        pt_off.append((o, o + ifree[m]))
        o += 2 * ifree[m]
    pt_cols = o  # 36864

    mid = ExitStack()
    try:
        with ExitStack() as p1s:
            qtp = p1s.enter_context(tc.tile_pool(name="qt", bufs=1))
            qt = qtp.tile([128, NDT, 1024], BF16)

            p1 = ExitStack()
            tpp = p1.enter_context(tc.tile_pool(name="tp_ps", bufs=4, space="PSUM"))
            prp = p1.enter_context(tc.tile_pool(name="prj_ps", bufs=3, space="PSUM"))
            natp = p1.enter_context(tc.tile_pool(name="nat", bufs=3))
            stg = p1.enter_context(tc.tile_pool(name="stg", bufs=4))

            def transpose_block(src_dram, dst, jt, q=None):
                nat = natp.tile([128, D], F32, tag="nat")
                (q or nc.sync).dma_start(nat, src_dram[jt * 128 : (jt + 1) * 128, :])
                for dt in range(NDT):
                    tp = tpp.tile([128, 128], F32, tag="tp")
                    nc.tensor.transpose(
                        tp, nat[:, dt * 128 : (dt + 1) * 128], ident
                    )
                    nc.vector.tensor_copy(
                        dst[:, dt, jt * 128 : (jt + 1) * 128], tp
                    )

            def transpose_in(src_dram, dst):
                for jt in range(8):
                    transpose_block(src_dram, dst, jt)

            with ExitStack() as zs:
                ztp = zs.enter_context(tc.tile_pool(name="zt", bufs=1))
                zt = ztp.tile([128, NDT, 1024], F32R)
                # transpose only z rows 0..511 up front; K-proj's jb=0 chains
                # need just those, so the K gather starts ~25us earlier.  The
                # second half transposes are injected between the first two
                # chains.
                for jt in range(4):
                    transpose_block(z, zt, jt)

                # K^T = Wk^T @ z^T, bf16 chunked layout for the gather
                with ExitStack() as ks:
                  wpp = ks.enter_context(tc.tile_pool(name="wpanel", bufs=4))
                  wvp = ks.enter_context(tc.tile_pool(name="wvp", bufs=2))
                  for t in range(NDT):
                    wp = wpp.tile([128, NDT, 128], F32R, tag="wp")
                    nc.sync.dma_start(
                        wp,
                        wk[:, t * 128 : (t + 1) * 128]
                        .rearrange("(dt p) c -> p dt c", p=128)
                        .bitcast(F32R),
                    )
                    for jb in range(2):
                        ps = prp.tile([128, 512], F32, tag="prj")
                        for dt in range(NDT):
                            nc.tensor.matmul(
                                ps,
                                wp[:, dt, :],
                                zt[:, dt, jb * 512 : (jb + 1) * 512],
                                start=(dt == 0),
                                stop=(dt == NDT - 1),
                            )
                        st = stg.tile([128, 512], BF16, tag="stg")
                        if zero_bias:
                            nc.scalar.activation(
                                st, ps, mybir.ActivationFunctionType.Copy
                            )
                        else:
                            nc.scalar.activation(
                                st,
                                ps,
                                mybir.ActivationFunctionType.Identity,
                                bias=bk_sb[:, t : t + 1],
                            )
                        # gpsimd queue: the stores wait on their activations,
                        # which would head-block the SP load queue
                        for h in range(2):
                            nc.gpsimd.dma_start(
                                kt_loc[2 * jb + h][:, t, :],
                                st[:, h * JC : (h + 1) * JC],
                            )
                        if t == 0 and jb == 0:
                            for jt in range(4, 8):
                                transpose_block(z, zt, jt)
                  nc.gpsimd.collective_compute(
                      "AllGather",
                      mybir.AluOpType.bypass,
                      replica_groups=[list(range(NCORES))],
                      ins=[kt_loc.ap().opt()],
                      outs=[kt_g.ap().opt()],
                  )

                  # V = z @ Wv (natural layout), bias folded into the output
                  for h in range(8):
                    wvh = wvp.tile([128, NDT, 256], F32R, tag="wvh")
                    nc.sync.dma_start(
                        wvh,
                        wv[:, h * 256 : (h + 1) * 256]
                        .rearrange("(dt p) c -> p dt c", p=128)
                        .bitcast(F32R),
                    )
                    for jt in range(8):
                        ps = prp.tile([128, 256], F32, tag="prj")
                        for dt in range(NDT):
                            nc.tensor.matmul(
                                ps,
                                zt[:, dt, jt * 128 : (jt + 1) * 128],
                                wvh[:, dt, :],
                                start=(dt == 0),
                                stop=(dt == NDT - 1),
                            )
                        st = stg.tile([128, 256], BF16, tag="stg")
                        nc.vector.tensor_copy(st, ps)
                        nc.gpsimd.dma_start(
                            v_loc[
                                jt * 128 : (jt + 1) * 128,
                                h * 256 : (h + 1) * 256,
                            ],
                            st,
                        )

            with ExitStack() as xs:
                xtp = xs.enter_context(tc.tile_pool(name="xt", bufs=1))
                xt = xtp.tile([128, NDT, 1024], F32R)
                wpx = xs.enter_context(tc.tile_pool(name="wpx", bufs=2))
                transpose_in(x, xt)
                # Q^T = Wq^T @ x^T, straight into SBUF (bf16)
                for t in range(NDT):
                    wp = wpx.tile([128, NDT, 128], F32R, tag="wp")
                    nc.sync.dma_start(
                        wp,
                        wq[:, t * 128 : (t + 1) * 128]
                        .rearrange("(dt p) c -> p dt c", p=128)
                        .bitcast(F32R),
                    )
                    for jb in range(2):
                        ps = prp.tile([128, 512], F32, tag="prj")
                        for dt in range(NDT):
                            nc.tensor.matmul(
                                ps,
                                wp[:, dt, :],
                                xt[:, dt, jb * 512 : (jb + 1) * 512],
                                start=(dt == 0),
                                stop=(dt == NDT - 1),
                            )
                        if zero_bias:
                            nc.scalar.activation(
                                qt[:, t, jb * 512 : (jb + 1) * 512],
                                ps,
                                mybir.ActivationFunctionType.Copy,
                            )
                        else:
                            nc.scalar.activation(
                                qt[:, t, jb * 512 : (jb + 1) * 512],
                                ps,
                                mybir.ActivationFunctionType.Identity,
                                bias=bq_sb[:, t : t + 1],
                            )

            p1.close()  # free phase-1 PSUM/SBUF pools before attention

            # ---------- Phase 2a: all causal scores -> pt stash ----------
            ptp = mid.enter_context(tc.tile_pool(name="pt", bufs=1))
            pt_all = ptp.tile([128, pt_cols], BF16)
            rcp = mid.enter_context(tc.tile_pool(name="rc", bufs=1))
            recip = rcp.tile([128, NSLOT, 2], F32)

            with ExitStack() as p2a:
                ktp = p2a.enter_context(tc.tile_pool(name="kt", bufs=3))
                stp = p2a.enter_context(
                    tc.tile_pool(name="st_ps", bufs=3, space="PSUM")
                )
                llp = p2a.enter_context(
                    tc.tile_pool(name="l_ps", bufs=1, space="PSUM")
                )
                mkp = p2a.enter_context(tc.tile_pool(name="mk", bufs=4))
                l_ps = llp.tile([128, NSLOT, 2], F32)

                for m in range(NCH):
                    g = m // 4
                    nf = ifree[m]
                    kt = ktp.tile([128, NDT, JC], BF16, tag="kt")
                    nc.sync.dma_start(kt, kt_g[m])
                    for jt in range(2):
                        st = stp.tile([128, 1024], F32, tag="st")
                        # moving free dim caps at 512 per matmul
                        for o in range(0, nf, 512):
                            w = min(512, nf - o)
                            for dt in range(NDT):
                                nc.tensor.matmul(
                                    st[:, o : o + w],
                                    kt[:, dt, jt * 128 : (jt + 1) * 128],
                                    qt[:, dt, 128 * g + o : 128 * g + o + w],
                                    start=(dt == 0),
                                    stop=(dt == NDT - 1),
                                )
                        off = pt_off[m][0] + jt * nf
                        pt = pt_all[:, off : off + nf]
                        nc.scalar.activation(
                            pt, st[:, :nf], mybir.ActivationFunctionType.Exp,
                            scale=SCALE,
                        )
                        # diagonal-band mask: only slot g of this chunk can
                        # cross the diagonal for any core
                        k = 2 * m + jt
                        mk = mkp.tile([128, 128], BF16, tag="mk")
                        nc.vector.tensor_scalar(
                            mk,
                            ig_sb[:, g, :],
                            jg[:, k : k + 1],
                            None,
                            mybir.AluOpType.is_ge,
                        )
                        nc.vector.tensor_mul(pt[:, :128], pt[:, :128], mk)

                # AG(V) issued after the kt_g reads so the tile framework's
                # conservative collective ordering doesn't stall the score
                # loads behind it; it still starts the moment AG(K) releases
                # the collective channel.
                nc.gpsimd.collective_compute(
                    "AllGather",
                    mybir.AluOpType.bypass,
                    replica_groups=[list(range(NCORES))],
                    ins=[v_loc.ap().opt()],
                    outs=[v_g.ap().opt()],
                )

                # softmax denominators: 1-row ones-matmuls, PSUM-accumulated
                for s in range(NSLOT):
                    n = (4 * s + 4) * 2
                    i = 0
                    for m in range(4 * s + 4):
                        col = (s - m // 4) * 128
                        for jt in range(2):
                            off = pt_off[m][0] + jt * ifree[m]
                            nc.tensor.matmul(
                                l_ps[:, s, :],
                                pt_all[:, off + col : off + col + 128],
                                ones,
                                start=(i == 0),
                                stop=(i == n - 1),
                            )
                            i += 1
                nc.vector.reciprocal(recip, l_ps)

            # ---------- Phase 2b: P^T @ V by chunk groups, dv halves ----------
            with ExitStack() as p2b:
                accp = p2b.enter_context(tc.tile_pool(name="acc", bufs=1))
                vcp = p2b.enter_context(tc.tile_pool(name="vc", bufs=10))
                pvp = p2b.enter_context(
                    tc.tile_pool(name="pv_ps", bufs=8, space="PSUM")
                )

                for half in range(2):
                    hof = half * 1024
                    acc = accp.tile([128, NSLOT, 1024], F32, tag="acc")
                    for g in range(NSLOT):
                        vts = []
                        for mi in range(4):
                            m = 4 * g + mi
                            vc = vcp.tile([128, 2, 1024], BF16, tag="vc")
                            # gpsimd queue: keeps these (gated on AG-V) off the
                            # SP DMA queue so kt loads are never stuck behind
                            nc.gpsimd.dma_start(
                                vc,
                                v_g[m * JC : (m + 1) * JC, hof : hof + 1024]
                                .rearrange("(jt p) d -> p jt d", p=128),
                            )
                            vts.append(vc)
                        for s in range(g, NSLOT):
                            col = (s - g) * 128
                            for dvc in range(2):
                                pv = pvp.tile([128, 512], F32, tag="pv")
                                for mi in range(4):
                                    m = 4 * g + mi
                                    for jt in range(2):
                                        off = pt_off[m][0] + jt * ifree[m]
                                        nc.tensor.matmul(
                                            pv,
                                            pt_all[:, off + col : off + col + 128],
                                            vts[mi][
                                                :, jt, dvc * 512 : (dvc + 1) * 512
                                            ],
                                            start=(mi == 0 and jt == 0),
                                            stop=(mi == 3 and jt == 1),
                                        )
                                dst = acc[:, s, dvc * 512 : (dvc + 1) * 512]
                                if g == 0:
                                    nc.vector.tensor_copy(dst, pv)
                                else:
                                    nc.vector.tensor_add(dst, dst, pv)
                        # slot g's half is final after its group-g flush
                        fin = acc[:, g, :]
                        nc.scalar.activation(
                            fin, fin, mybir.ActivationFunctionType.Copy,
                            scale=recip[:, g, 0:1],
                        )
                        if not zero_bias:
                            nc.vector.tensor_add(
                                fin, fin, bv_bc[:, hof : hof + 1024]
                            )
                        nc.sync.dma_start(
                            out[g * 128 : (g + 1) * 128, hof : hof + 1024], fin
                        )
            mid.close()  # release pt/rc before p1s releases qt (stack order)
    finally:
        mid.close()


def make_in_maps(inputs):
    x = np.ascontiguousarray(np.asarray(inputs["x"], dtype=np.float32))
    z = np.ascontiguousarray(np.asarray(inputs["z"], dtype=np.float32))
    in_maps = []
    iota = np.arange(128, dtype=np.float32)
    for c in range(NCORES):
        rows = [128 * (8 * s + c) for s in range(NSLOT)]
        x_rows = np.concatenate([x[r : r + 128] for r in rows], axis=0)
        ig = np.concatenate([r + iota for r in rows])
        in_maps.append(
            {
                "x_rows": np.ascontiguousarray(x_rows),
                "z_blk": np.ascontiguousarray(z[c * 1024 : (c + 1) * 1024]),
                "wq": np.asarray(inputs["Wq"], dtype=np.float32),
                "wk": np.asarray(inputs["Wk"], dtype=np.float32),
                "wv": np.asarray(inputs["Wv"], dtype=np.float32),
                "bq": np.asarray(inputs["bq"], dtype=np.float32),
                "bk": np.asarray(inputs["bk"], dtype=np.float32),
                "bv": np.asarray(inputs["bv"], dtype=np.float32),
                "ig_rows": np.ascontiguousarray(ig),
            }
        )
    return in_maps


def assemble(results):
    full = np.empty((L, D), dtype=np.float32)
    for c in range(NCORES):
        o = results[c]["out"]
        for s in range(NSLOT):
            r = 128 * (8 * s + c)
            full[r : r + 128] = o[s * 128 : (s + 1) * 128]
    return full


def kernel(x, z, Wq, bq, Wk, bk, Wv, bv):
    zb = not (np.any(bq) or np.any(bk) or np.any(bv))
    if _cache.get("zb") != zb:
        _cache.pop("nc", None)
    if "nc" not in _cache:
        t0 = time.time()
        _cache["nc"] = _build(zero_bias=zb)
        _cache["zb"] = zb
        _cache["build_s"] = time.time() - t0

    in_maps = make_in_maps(
        {"x": x, "z": z, "Wq": Wq, "bq": bq, "Wk": Wk, "bk": bk, "Wv": Wv, "bv": bv}
    )

    t0 = time.time()
    last_err = None
    for attempt in range(3):
        try:
            res = run_bass_kernel_spmd(
                _cache["nc"], in_maps, core_ids=list(range(NCORES))
            )
            break
        except Exception as e:  # transient NRT_EXEC_UNIT_UNRECOVERABLE after a
            last_err = e  # prior process exits; an immediate retry succeeds
            time.sleep(10)
    else:
        raise last_err
    _cache["run_s"] = time.time() - t0

    return assemble(res.results)


def timed_run(in_maps, n_iter=3, pipelined=False):
    """Stage inputs on the 8 cores, run the kernel n_iter times, return
    (per-core results, list of wall seconds per on-device invocation)."""
    import jax
    import jax.numpy as jnp
    from jax.experimental.shard_map import shard_map
    from jax.sharding import Mesh, NamedSharding, PartitionSpec

    from concourse import mybir as _mb
    from concourse.bass2jax import (
        _bass_exec_p,
        install_neuronx_cc_hook,
        partition_id_tensor,
    )

    nc = _cache["nc"]
    install_neuronx_cc_hook()

    partition_name = nc.partition_id_tensor.name if nc.partition_id_tensor else None
    in_names, out_names, out_avals, zero_outs = [], [], [], []
    for alloc in nc.m.functions[0].allocations:
        if not isinstance(alloc, _mb.MemoryLocationSet):
            continue
        name = alloc.memorylocations[0].name
        if alloc.kind == "ExternalInput":
            if name != partition_name:
                in_names.append(name)
        elif alloc.kind == "ExternalOutput":
            out_names.append(name)
            out_avals.append(
                jax.core.ShapedArray(tuple(alloc.tensor_shape), _mb.dt.np(alloc.dtype))
            )
            zero_outs.append(
                np.zeros(tuple(alloc.tensor_shape), _mb.dt.np(alloc.dtype))
            )
    n_params = len(in_names)
    n_outs = len(out_names)
    all_in_names = list(in_names) + out_names
    if partition_name is not None:
        all_in_names.append(partition_name)
    donate = tuple(range(n_params, n_params + n_outs))

    def _body(*args):
        operands = list(args)
        if partition_name is not None:
            operands.append(partition_id_tensor())
        outs = _bass_exec_p.bind(
            *operands,
            out_avals=tuple(out_avals),
            in_names=tuple(all_in_names),
            out_names=tuple(out_names),
            lowering_input_output_aliases=(),
            sim_require_finite=True,
            sim_require_nnan=True,
            nc=nc,
        )
        return tuple(outs)

    devices = jax.devices()[:NCORES]
    mesh = Mesh(np.asarray(devices), ("core",))
    spec = NamedSharding(mesh, PartitionSpec("core"))
    sharded = jax.jit(
        shard_map(
            _body,
            mesh=mesh,
            in_specs=(PartitionSpec("core"),) * (n_params + n_outs),
            out_specs=(PartitionSpec("core"),) * n_outs,
            check_rep=False,
        ),
        donate_argnums=donate,
        keep_unused=True,
    )

    concat_in = [
        jax.device_put(
            np.concatenate([np.asarray(in_maps[c][n]) for c in range(NCORES)], axis=0),
            spec,
        )
        for n in in_names
    ]
    zero_sets = [
        [
            jax.device_put(
                np.zeros((NCORES * z.shape[0], *z.shape[1:]), z.dtype), spec
            )
            for z in zero_outs
        ]
        for _ in range(n_iter)
    ]
    for a in concat_in:
        a.block_until_ready()
    for zs in zero_sets:
        for z in zs:
            z.block_until_ready()

    times = []
    out_arrs = None
    for it in range(n_iter):
        t0 = time.time()
        out_arrs = sharded(*concat_in, *zero_sets[it])
        for o in out_arrs:
            o.block_until_ready()
        times.append(time.time() - t0)

    # pipelined launches: amortize the per-dispatch tunnel overhead
    def fresh_zero_sets(k):
        zs = [
            [
                jax.device_put(
                    np.zeros((NCORES * z.shape[0], *z.shape[1:]), z.dtype), spec
                )
                for z in zero_outs
            ]
            for _ in range(k)
        ]
        for zset in zs:
            for zz in zset:
                zz.block_until_ready()
        return zs

    for k in ((2, 8) if pipelined else ()):
        zsets = fresh_zero_sets(k)
        t0 = time.time()
        outs = [sharded(*concat_in, *zsets[i]) for i in range(k)]
        for oset in outs:
            for o in oset:
                o.block_until_ready()
        times.append((k, time.time() - t0))

    results = [
        {
            n: np.asarray(out_arrs[i]).reshape(NCORES, *out_avals[i].shape)[c]
            for i, n in enumerate(out_names)
        }
        for c in range(NCORES)
    ]
    return results, times
